# revision 32
# baseline (speedup 1.0000x reference)
"""Distributed Bass kernel for nn_Attention_75514114998541.

GQA attention block (16 Q heads / 4 KV heads, head_dim 128, hidden 2048,
B=2, S=2048) with per-head RMSNorm on q/k, causal softmax, output proj.

Sharding: 8 cores = 2 (batch) x 4 (head groups). Core 4*b+g handles batch b
and heads [4g, 4g+4) (= kv head g). Wq/Wk/Wv column-sharded, Wo row-sharded;
each core emits a partial [S, HID] output, host sums the 4 partials per batch.

All matmuls run in float32r (full PE rate at N>=256, ~1e-3 rounding).
Everything stays in feature-on-partition layout:
  xT[hid, tok] (PE transposes) -> QT/KT[d, tok] -> ST[k, q] -> PT -> OT[d, q]
  -> out[tok, hid].
Per-token row scales (rmsnorm 1/rms, softmax 1/denom) are [1, 512] rows
(ones-vector matmul column-sums) broadcast across partitions with
gpsimd.partition_broadcast.
"""
import contextlib
import ctypes
import os
import sys
import types

import numpy as np

sys.path.insert(0, "/opt/trn_rl_repo")

import concourse.bacc as bacc
import concourse.mybir as mybir
import concourse.tile as tile
from concourse.bass_utils import run_bass_kernel_spmd

F32 = mybir.dt.float32
F32R = mybir.dt.float32r

NCORES = 8
S = 2048            # sequence length (= tokens per batch)
HID = 2048          # hidden dim
D = 128             # head dim
HQ = 4              # q heads per core
STRIP = 512         # token strip (matmul moving free dim)
NSTRIP = S // STRIP          # 4
KT = HID // 128              # 16 hidden k-tiles
TOKT = S // 128              # 16 token 128-blocks
EPS = 1e-6
NEG = -1.0e30
TRACE = os.environ.get("BASS_KERNEL_TRACE", "0") == "1"


def _install_profile_shim():
    """antenv.axon_hooks shim so trace=True captures NTFF under axon."""
    if "antenv.axon_hooks" in sys.modules:
        return
    so_path = "/opt/axon/libaxon_pjrt.so"
    try:
        lib = ctypes.CDLL(so_path)
    except OSError:
        return
    if not hasattr(lib, "axon_start_nrt_profile"):
        return
    lib.axon_start_nrt_profile.argtypes = [ctypes.POINTER(ctypes.c_int64), ctypes.c_size_t]
    lib.axon_start_nrt_profile.restype = ctypes.c_int64
    lib.axon_stop_nrt_profile.argtypes = [ctypes.c_char_p]
    lib.axon_stop_nrt_profile.restype = ctypes.c_int64

    @contextlib.contextmanager
    def _hook(output_dir, device_ids):
        import jax

        jax.devices()
        if device_ids:
            ids = (ctypes.c_int64 * len(device_ids))(*device_ids)
            rc = lib.axon_start_nrt_profile(ids, len(device_ids))
        else:
            rc = lib.axon_start_nrt_profile(None, 0)
        if rc != 0:
            raise RuntimeError(f"axon_start_nrt_profile rc={rc}")
        try:
            yield
        finally:
            n = lib.axon_stop_nrt_profile(str(output_dir).encode())
            if n < 0:
                raise RuntimeError(f"axon_stop_nrt_profile rc={n}")

    mod = types.ModuleType("antenv.axon_hooks")
    state = {"hook": _hook}
    mod.set_axon_ntff_profile_hook = lambda h: state.update(hook=h)
    mod.get_axon_ntff_profile_hook = lambda: state["hook"]
    sys.modules["antenv.axon_hooks"] = mod
    try:
        import antenv

        antenv.axon_hooks = mod
    except ImportError:
        pass


def build():
    nc = bacc.Bacc("TRN2", target_bir_lowering=False, debug=False, num_devices=NCORES)

    xt_ext = nc.dram_tensor("xt", [HID, S], F32, kind="ExternalInput")
    wq_ext = nc.dram_tensor("wq", [HID, HQ * D], F32, kind="ExternalInput")
    wk_ext = nc.dram_tensor("wk", [HID, D], F32, kind="ExternalInput")
    wv_ext = nc.dram_tensor("wv", [HID, D], F32, kind="ExternalInput")
    wo_ext = nc.dram_tensor("wo", [HQ * D, HID], F32, kind="ExternalInput")
    gq_ext = nc.dram_tensor("gq", [D, 1], F32, kind="ExternalInput")
    gk_ext = nc.dram_tensor("gk", [D, 1], F32, kind="ExternalInput")
    masks_ext = nc.dram_tensor("masks", [4, 128, STRIP], F32, kind="ExternalInput")
    ones_ext = nc.dram_tensor("ones", [128, 1], F32, kind="ExternalInput")
    ident_ext = nc.dram_tensor("ident", [128, 128], F32, kind="ExternalInput")
    out_ext = nc.dram_tensor("out", [S, HID], F32, kind="ExternalOutput")

    Exp = mybir.ActivationFunctionType.Exp
    Sqrt = mybir.ActivationFunctionType.Sqrt
    Square = mybir.ActivationFunctionType.Square
    mult = mybir.AluOpType.mult
    add = mybir.AluOpType.add
    scale_qk = float(D) ** -0.5

    with tile.TileContext(nc) as tc, contextlib.ExitStack() as ctx:
        wpool = ctx.enter_context(tc.tile_pool(name="w", bufs=1))
        cpool = ctx.enter_context(tc.tile_pool(name="c", bufs=1))
        xtp = ctx.enter_context(tc.tile_pool(name="xt", bufs=20))
        kvp = ctx.enter_context(tc.tile_pool(name="kv", bufs=1))
        qtp = ctx.enter_context(tc.tile_pool(name="qt", bufs=4))
        otp = ctx.enter_context(tc.tile_pool(name="ot", bufs=8))
        ptp = ctx.enter_context(tc.tile_pool(name="pt", bufs=6))
        scr = ctx.enter_context(tc.tile_pool(name="scr", bufs=2))
        rowp = ctx.enter_context(tc.tile_pool(name="rows", bufs=6))
        bcp = ctx.enter_context(tc.tile_pool(name="bc", bufs=4))
        outp = ctx.enter_context(tc.tile_pool(name="outev", bufs=2))
        accp = ctx.enter_context(tc.tile_pool(name="accp", bufs=2))
        bigps = ctx.enter_context(tc.tile_pool(name="bigps", bufs=4, space="PSUM"))
        stps = ctx.enter_context(tc.tile_pool(name="stps", bufs=2, space="PSUM"))
        otps = ctx.enter_context(tc.tile_pool(name="otps", bufs=1, space="PSUM"))
        rowps = ctx.enter_context(tc.tile_pool(name="rowps", bufs=1, space="PSUM"))

        # ---- weights: xt strip 0 + per-k weights first (so matmuls start early),
        # wo + ident deferred below.
        xt0 = []
        wq_t, wk_t, wv_t = [], [], []
        for k in range(KT):
            xt_k = xtp.tile([128, STRIP], F32R, name=f"xt0_{k}", tag="xt")
            nc.sync.dma_start(out=xt_k[:], in_=xt_ext[k * 128:(k + 1) * 128, 0:STRIP].bitcast(F32R))
            xt0.append(xt_k)
            wq_k = wpool.tile([128, HQ * D], F32R, name=f"wq{k}", tag=f"wq{k}")
            nc.sync.dma_start(out=wq_k[:], in_=wq_ext[k * 128:(k + 1) * 128, :].bitcast(F32R))
            wk_k = wpool.tile([128, D], F32R, name=f"wk{k}", tag=f"wk{k}")
            nc.sync.dma_start(out=wk_k[:], in_=wk_ext[k * 128:(k + 1) * 128, :].bitcast(F32R))
            wv_k = wpool.tile([128, D], F32R, name=f"wv{k}", tag=f"wv{k}")
            nc.sync.dma_start(out=wv_k[:], in_=wv_ext[k * 128:(k + 1) * 128, :].bitcast(F32R))
            wq_t.append(wq_k)
            wk_t.append(wk_k)
            wv_t.append(wv_k)
        gq_sb = cpool.tile([D, 1], F32, name="gq_sb", tag="gq_sb")
        nc.sync.dma_start(out=gq_sb[:], in_=gq_ext[:])
        gk_sb = cpool.tile([D, 1], F32, name="gk_sb", tag="gk_sb")
        nc.sync.dma_start(out=gk_sb[:], in_=gk_ext[:])
        tri_sb = cpool.tile([128, 128], F32R, name="tri_sb", tag="tri_sb")
        nc.sync.dma_start(out=tri_sb[:], in_=masks_ext[0, :, :128].bitcast(F32R))
        ones_sb = cpool.tile([128, 1], F32R, name="ones_sb", tag="ones_sb")
        nc.sync.dma_start(out=ones_sb[:], in_=ones_ext[:].bitcast(F32R))
        ident_sb = cpool.tile([128, 128], F32R, name="ident_sb", tag="ident_sb")
        nc.sync.dma_start(out=ident_sb[:], in_=ident_ext[:].bitcast(F32R))
        wo_t = []
        for h in range(HQ):
            wo_h = wpool.tile([128, HID], F32R, name=f"wo{h}", tag=f"wo{h}")
            nc.sync.dma_start(out=wo_h[:], in_=wo_ext[h * 128:(h + 1) * 128, :].bitcast(F32R))
            wo_t.append(wo_h)
        eps_sb = cpool.tile([1, 1], F32, name="eps_sb", tag="eps_sb")
        nc.vector.memset(eps_sb[:], EPS)

        kt_strips = []  # K-hat-T strips [128 d, STRIP tok], persistent
        v_tiles = []    # V tiles [128 tok, D], persistent
        pending_out = []

        def emit_out_proj(chunk=None):
            # chunk=None -> emit everything pending; chunk=tb -> emit one
            # tb-column of the oldest pending strip
            while pending_out:
                ps_, heads_ = pending_out[0]
                tbs = range(4) if chunk is None else [chunk]
                for tb in tbs:
                    tok0 = ps_ * STRIP + tb * 128
                    for hs in range(4):
                        op_ps = bigps.tile([128, STRIP], F32, name=f"op{ps_}_{tb}_{hs}", tag="bigps")
                        for h in range(HQ):
                            nc.tensor.matmul(
                                op_ps[:],
                                heads_[h][:, tb * 128:(tb + 1) * 128],
                                wo_t[h][:, hs * STRIP:(hs + 1) * STRIP],
                                start=(h == 0), stop=(h == HQ - 1),
                            )
                        ob = outp.tile([128, STRIP], F32, name=f"ob{ps_}_{tb}_{hs}", tag="ob")
                        nc.scalar.copy(ob[:], op_ps[:])
                        nc.sync.dma_start(
                            out=out_ext[tok0:tok0 + 128, hs * STRIP:(hs + 1) * STRIP],
                            in_=ob[:],
                        )
                if chunk is not None:
                    return
                pending_out.pop(0)

        def norm_row_scale(raw_ps, suffix):
            """[128, STRIP] psum -> [128, STRIP] bcast of 1/rms over partitions."""
            sq = scr.tile([128, STRIP], F32R, name=f"sq_{suffix}", tag="sq")
            nc.scalar.activation(sq[:], raw_ps[:], Square)
            ss_ps = rowps.tile([1, STRIP], F32, name=f"ss_{suffix}", tag="rowps")
            nc.tensor.matmul(ss_ps[:], ones_sb[:], sq[:], start=True, stop=True)
            rms = rowp.tile([1, STRIP], F32, name=f"rms_{suffix}", tag="rows")
            nc.scalar.activation(rms[:], ss_ps[:], Sqrt, bias=eps_sb[:], scale=1.0 / D)
            bcr = bcp.tile([128, STRIP], F32, name=f"bcr_{suffix}", tag="bc")
            nc.gpsimd.partition_broadcast(bcr[:], rms[:])
            bc = bcp.tile([128, STRIP], F32, name=f"bc_{suffix}", tag="bc")
            nc.vector.reciprocal_approx_fast(bc[:], bcr[:])
            return bc

        for s in range(NSTRIP):
            # ---- 1) load xT strip tiles [128 hid, STRIP tok] (x pre-transposed on host)
            tsl = slice(s * STRIP, (s + 1) * STRIP)
            if s == 0:
                xt = xt0
            else:
                xt = []
                for k in range(KT):
                    xt_k = xtp.tile([128, STRIP], F32R, name=f"xt{s}_{k}", tag="xt")
                    nc.sync.dma_start(out=xt_k[:], in_=xt_ext[k * 128:(k + 1) * 128, tsl].bitcast(F32R))
                    xt.append(xt_k)

            # ---- 2) projections: Q0, Q1, K, Q2, Q3, V. K's norm chain hides
            # under Q2/Q3 matmuls; V's matmuls + transposes cover Q3's chain.
            qt_h = [None] * HQ

            def project_q(h):
                qraw = bigps.tile([128, STRIP], F32, name=f"qraw{s}_{h}", tag="bigps")
                for k in range(KT):
                    nc.tensor.matmul(
                        qraw[:], wq_t[k][:, h * D:(h + 1) * D], xt[k][:],
                        start=(k == 0), stop=(k == KT - 1),
                    )
                bcq = norm_row_scale(qraw, f"q{s}_{h}")
                qn = qtp.tile([128, STRIP], F32R, name=f"qt{s}_{h}", tag="qt")
                nc.vector.scalar_tensor_tensor(qn[:], qraw[:], gq_sb[:], bcq[:], mult, mult)
                qt_h[h] = qn

            project_q(0)
            project_q(1)
            project_q(2)
            project_q(3)

            kraw = bigps.tile([128, STRIP], F32, name=f"kraw{s}", tag="bigps")
            for k in range(KT):
                nc.tensor.matmul(
                    kraw[:], wk_t[k][:], xt[k][:],
                    start=(k == 0), stop=(k == KT - 1),
                )
            bck = norm_row_scale(kraw, f"k{s}")
            kn = kvp.tile([128, STRIP], F32R, name=f"kt_strip{s}", tag="kt", bufs=NSTRIP)
            nc.vector.scalar_tensor_tensor(kn[:], kraw[:], gk_sb[:], bck[:], mult, mult)
            kt_strips.append(kn)

            vraw = bigps.tile([128, STRIP], F32, name=f"vraw{s}", tag="bigps")
            for k in range(KT):
                nc.tensor.matmul(
                    vraw[:], wv_t[k][:], xt[k][:],
                    start=(k == 0), stop=(k == KT - 1),
                )
            vt_sb = scr.tile([128, STRIP], F32R, name=f"vt_sb{s}", tag="sq")
            nc.vector.tensor_copy(vt_sb[:], vraw[:])
            for tb in range(4):
                tp = bigps.tile([128, 128], F32R, name=f"vtp{s}_{tb}", tag="bigps")
                nc.tensor.transpose(tp[:], vt_sb[:, tb * 128:(tb + 1) * 128], ident_sb[:])
                vt = kvp.tile([128, D], F32R, name=f"v{s}_{tb}", tag="v", bufs=TOKT)
                nc.vector.tensor_copy(vt[:], tp[:])
                v_tiles.append(vt)

            emit_out_proj()

            # ---- 3) attention for q-strip s, heads 0..3
            # (out-proj chunks of the previous strip are interleaved between
            # heads to fill PE bubbles in the softmax-bound phase)
            nkt = 4 * s + 4  # causal: k-tiles 0 .. 4s+3
            ot_heads = []
            for h in range(HQ):
                ot_ps = otps.tile([128, STRIP], F32, name=f"ot{s}_{h}", tag="otps")
                acc = accp.tile([128, STRIP], F32R, name=f"acc{s}_{h}", tag="acc")
                pts = [None] * nkt

                def issue_st(k, h=h, pts=pts):
                    st_ps = stps.tile([128, STRIP], F32, name=f"st{s}_{h}_{k}", tag="stps")
                    nc.tensor.matmul(
                        st_ps[:],
                        kt_strips[k // 4][:, (k % 4) * 128:(k % 4 + 1) * 128],
                        qt_h[h][:],
                        start=True, stop=True,
                    )
                    pt = ptp.tile([128, STRIP], F32R, name=f"pt{s}_{h}_{k}", tag="pt")
                    j = k - 4 * s
                    if j < 0:
                        nc.scalar.activation(pt[:], st_ps[:], Exp, scale=scale_qk)
                    else:
                        c0 = 128 * j
                        if c0 > 0:
                            nc.gpsimd.memset(pt[:, :c0].bitcast(F32), 0.0)
                        nc.scalar.activation(pt[:, c0:], st_ps[:, c0:], Exp, scale=scale_qk)
                        nc.vector.tensor_tensor(
                            pt[:, c0:c0 + 128], pt[:, c0:c0 + 128], tri_sb[:], mult
                        )
                    pts[k] = pt

                def issue_pv(k, ot_ps=ot_ps, acc=acc, pts=pts, nkt=nkt):
                    nc.tensor.matmul(
                        ot_ps[:], v_tiles[k][:], pts[k][:],
                        start=(k == 0), stop=(k == nkt - 1),
                    )
                    if k == 0:
                        nc.vector.tensor_copy(acc[:], pts[k][:])
                    else:
                        nc.vector.tensor_add(acc[:], acc[:], pts[k][:])

                # software-pipeline: ST(k+1) issued before PV(k)
                issue_st(0)
                for k in range(1, nkt):
                    issue_st(k)
                    issue_pv(k - 1)
                issue_pv(nkt - 1)

                # evict OT unnormalized immediately (frees the single otps bank),
                # normalize in place once the denominator row is ready
                ot_sb = otp.tile([128, STRIP], F32R, name=f"otsb{s}_{h}", tag="ot")
                nc.vector.tensor_copy(ot_sb[:], ot_ps[:])
                den_ps = rowps.tile([1, STRIP], F32, name=f"den{s}_{h}", tag="rowps")
                nc.tensor.matmul(den_ps[:], ones_sb[:], acc[:], start=True, stop=True)
                dstage = rowp.tile([1, STRIP], F32, name=f"dr{s}_{h}", tag="rows")
                nc.vector.tensor_copy(dstage[:], den_ps[:])
                bcd = bcp.tile([128, STRIP], F32, name=f"dbcr{s}_{h}", tag="bc")
                nc.gpsimd.partition_broadcast(bcd[:], dstage[:])
                bc = bcp.tile([128, STRIP], F32, name=f"dbc{s}_{h}", tag="bc")
                nc.vector.reciprocal_approx_fast(bc[:], bcd[:])
                nc.vector.tensor_tensor(ot_sb[:], ot_sb[:], bc[:], mult)
                ot_heads.append(ot_sb)

            # ---- 4) output projection deferred into the next strip (fills the
            # softmax-tail PE bubble); emitted by emit_out_proj below.
            pending_out.append((s, ot_heads))

        emit_out_proj()

    nc.compile()
    return nc


_NC_CACHE = None
last_result = None


def _masks_np():
    # masks[0, :, :128] = lower-triangle 0/1 validity (kr <= qc)
    m = np.zeros((4, 128, STRIP), np.float32)
    kr = np.arange(128)[:, None]
    qc = np.arange(STRIP)[None, :]
    m[0] = np.where(kr <= qc, 1.0, 0.0)
    return m


def kernel(x, Wq, Wk, Wv, Wo, gq, gk):
    global _NC_CACHE, last_result
    x = np.asarray(x, np.float32)
    Wq = np.asarray(Wq, np.float32)
    Wk = np.asarray(Wk, np.float32)
    Wv = np.asarray(Wv, np.float32)
    Wo = np.asarray(Wo, np.float32)
    gq = np.asarray(gq, np.float32)
    gk = np.asarray(gk, np.float32)

    masks = _masks_np()
    ones = np.ones((128, 1), np.float32)
    ident = np.eye(128, dtype=np.float32)
    in_maps = []
    for core in range(NCORES):
        b, g = core // 4, core % 4
        in_maps.append({
            "xt": np.ascontiguousarray(x[b].T),
            "wq": np.ascontiguousarray(Wq[:, g * HQ * D:(g + 1) * HQ * D]),
            "wk": np.ascontiguousarray(Wk[:, g * D:(g + 1) * D]),
            "wv": np.ascontiguousarray(Wv[:, g * D:(g + 1) * D]),
            "wo": np.ascontiguousarray(Wo[g * HQ * D:(g + 1) * HQ * D, :]),
            "gq": np.ascontiguousarray(gq.reshape(D, 1)),
            "gk": np.ascontiguousarray(gk.reshape(D, 1)),
            "masks": masks,
            "ones": ones,
            "ident": ident,
        })

    if TRACE:
        _install_profile_shim()
    if _NC_CACHE is None:
        _NC_CACHE = build()
    last_result = run_bass_kernel_spmd(
        _NC_CACHE, in_maps, core_ids=list(range(NCORES)), trace=TRACE
    )
    out = np.zeros((2, S, HID), np.float32)
    for core in range(NCORES):
        out[core // 4] += last_result.results[core]["out"]
    return out


# revision 43
# speedup vs baseline: 1.0649x; 1.0649x over previous
"""Distributed Bass kernel for nn_Attention_75514114998541.

GQA attention block (16 Q heads / 4 KV heads, head_dim 128, hidden 2048,
B=2, S=2048) with per-head RMSNorm on q/k, causal softmax, output proj.

Sharding: 8 cores = 2 (batch) x 4 (head groups). Core 4*b+g handles batch b
and heads [4g, 4g+4) (= kv head g). Wq/Wk/Wv column-sharded, Wo row-sharded;
each core emits a partial [S, HID] output, host sums the 4 partials per batch.

All matmuls run in float32r (full PE rate at N>=256, ~1e-3 rounding).
Everything stays in feature-on-partition layout:
  xT[hid, tok] (PE transposes) -> QT/KT[d, tok] -> ST[k, q] -> PT -> OT[d, q]
  -> out[tok, hid].
Per-token row scales (rmsnorm 1/rms, softmax 1/denom) are [1, 512] rows
(ones-vector matmul column-sums) broadcast across partitions with
gpsimd.partition_broadcast.
"""
import contextlib
import ctypes
import os
import sys
import types

import numpy as np

sys.path.insert(0, "/opt/trn_rl_repo")

import concourse.bacc as bacc
import concourse.mybir as mybir
import concourse.tile as tile
from concourse.bass_utils import run_bass_kernel_spmd

F32 = mybir.dt.float32
F32R = mybir.dt.float32r

NCORES = 8
S = 2048            # sequence length (= tokens per batch)
HID = 2048          # hidden dim
D = 128             # head dim
HQ = 4              # q heads per core
STRIP = 512         # token strip (matmul moving free dim)
NSTRIP = S // STRIP          # 4
KT = HID // 128              # 16 hidden k-tiles
TOKT = S // 128              # 16 token 128-blocks
EPS = 1e-6
TRACE = os.environ.get("BASS_KERNEL_TRACE", "0") == "1"


def _install_profile_shim():
    """antenv.axon_hooks shim so trace=True captures NTFF under axon."""
    if "antenv.axon_hooks" in sys.modules:
        return
    so_path = "/opt/axon/libaxon_pjrt.so"
    try:
        lib = ctypes.CDLL(so_path)
    except OSError:
        return
    if not hasattr(lib, "axon_start_nrt_profile"):
        return
    lib.axon_start_nrt_profile.argtypes = [ctypes.POINTER(ctypes.c_int64), ctypes.c_size_t]
    lib.axon_start_nrt_profile.restype = ctypes.c_int64
    lib.axon_stop_nrt_profile.argtypes = [ctypes.c_char_p]
    lib.axon_stop_nrt_profile.restype = ctypes.c_int64

    @contextlib.contextmanager
    def _hook(output_dir, device_ids):
        import jax

        jax.devices()
        if device_ids:
            ids = (ctypes.c_int64 * len(device_ids))(*device_ids)
            rc = lib.axon_start_nrt_profile(ids, len(device_ids))
        else:
            rc = lib.axon_start_nrt_profile(None, 0)
        if rc != 0:
            raise RuntimeError(f"axon_start_nrt_profile rc={rc}")
        try:
            yield
        finally:
            n = lib.axon_stop_nrt_profile(str(output_dir).encode())
            if n < 0:
                raise RuntimeError(f"axon_stop_nrt_profile rc={n}")

    mod = types.ModuleType("antenv.axon_hooks")
    state = {"hook": _hook}
    mod.set_axon_ntff_profile_hook = lambda h: state.update(hook=h)
    mod.get_axon_ntff_profile_hook = lambda: state["hook"]
    sys.modules["antenv.axon_hooks"] = mod
    try:
        import antenv

        antenv.axon_hooks = mod
    except ImportError:
        pass


def build():
    nc = bacc.Bacc("TRN2", target_bir_lowering=False, debug=False, num_devices=NCORES)

    xt_ext = nc.dram_tensor("xt", [HID, S], F32, kind="ExternalInput")
    wq_ext = nc.dram_tensor("wq", [HID, HQ * D], F32, kind="ExternalInput")
    wk_ext = nc.dram_tensor("wk", [HID, D], F32, kind="ExternalInput")
    wv_ext = nc.dram_tensor("wv", [HID, D], F32, kind="ExternalInput")
    wo_ext = nc.dram_tensor("wo", [HQ * D, HID], F32, kind="ExternalInput")
    gq_ext = nc.dram_tensor("gq", [D, 1], F32, kind="ExternalInput")
    gk_ext = nc.dram_tensor("gk", [D, 1], F32, kind="ExternalInput")
    masks_ext = nc.dram_tensor("masks", [4, 128, STRIP], F32, kind="ExternalInput")
    ones_ext = nc.dram_tensor("ones", [128, 1], F32, kind="ExternalInput")
    ident_ext = nc.dram_tensor("ident", [128, 128], F32, kind="ExternalInput")
    out_ext = nc.dram_tensor("out", [S, HID], F32, kind="ExternalOutput")

    Exp = mybir.ActivationFunctionType.Exp
    Sqrt = mybir.ActivationFunctionType.Sqrt
    Square = mybir.ActivationFunctionType.Square
    mult = mybir.AluOpType.mult
    scale_qk = float(D) ** -0.5

    with tile.TileContext(nc) as tc, contextlib.ExitStack() as ctx:
        wpool = ctx.enter_context(tc.tile_pool(name="w", bufs=1))
        cpool = ctx.enter_context(tc.tile_pool(name="c", bufs=1))
        xtp = ctx.enter_context(tc.tile_pool(name="xt", bufs=18))
        kvp = ctx.enter_context(tc.tile_pool(name="kv", bufs=1))
        qtp = ctx.enter_context(tc.tile_pool(name="qt", bufs=4))
        otp = ctx.enter_context(tc.tile_pool(name="ot", bufs=8))
        ptp = ctx.enter_context(tc.tile_pool(name="pt", bufs=8))
        scr = ctx.enter_context(tc.tile_pool(name="scr", bufs=2))
        rowp = ctx.enter_context(tc.tile_pool(name="rows", bufs=6))
        bcp = ctx.enter_context(tc.tile_pool(name="bc", bufs=4))
        outp = ctx.enter_context(tc.tile_pool(name="outev", bufs=2))
        accp = ctx.enter_context(tc.tile_pool(name="accp", bufs=2))
        bigps = ctx.enter_context(tc.tile_pool(name="bigps", bufs=4, space="PSUM"))
        stps = ctx.enter_context(tc.tile_pool(name="stps", bufs=2, space="PSUM"))
        otps = ctx.enter_context(tc.tile_pool(name="otps", bufs=1, space="PSUM"))
        rowps = ctx.enter_context(tc.tile_pool(name="rowps", bufs=1, space="PSUM"))

        # ---- weights: xt strip 0 + per-k weights first (so matmuls start early),
        # wo + ident deferred below.
        xt0 = []
        wq_t, wk_t, wv_t = [], [], []
        for k in range(KT):
            xt_k = xtp.tile([128, STRIP], F32R, name=f"xt0_{k}", tag="xt")
            nc.sync.dma_start(out=xt_k[:], in_=xt_ext[k * 128:(k + 1) * 128, 0:STRIP].bitcast(F32R))
            xt0.append(xt_k)
            wq_k = wpool.tile([128, HQ * D], F32R, name=f"wq{k}", tag=f"wq{k}")
            nc.sync.dma_start(out=wq_k[:], in_=wq_ext[k * 128:(k + 1) * 128, :].bitcast(F32R))
            wk_k = wpool.tile([128, D], F32R, name=f"wk{k}", tag=f"wk{k}")
            nc.sync.dma_start(out=wk_k[:], in_=wk_ext[k * 128:(k + 1) * 128, :].bitcast(F32R))
            wv_k = wpool.tile([128, D], F32R, name=f"wv{k}", tag=f"wv{k}")
            nc.sync.dma_start(out=wv_k[:], in_=wv_ext[k * 128:(k + 1) * 128, :].bitcast(F32R))
            wq_t.append(wq_k)
            wk_t.append(wk_k)
            wv_t.append(wv_k)
        gq_sb = cpool.tile([D, 1], F32, name="gq_sb", tag="gq_sb")
        nc.sync.dma_start(out=gq_sb[:], in_=gq_ext[:])
        gk_sb = cpool.tile([D, 1], F32, name="gk_sb", tag="gk_sb")
        nc.sync.dma_start(out=gk_sb[:], in_=gk_ext[:])
        tri_sb = cpool.tile([128, 128], F32R, name="tri_sb", tag="tri_sb")
        nc.sync.dma_start(out=tri_sb[:], in_=masks_ext[0, :, :128].bitcast(F32R))
        ones_sb = cpool.tile([128, 1], F32R, name="ones_sb", tag="ones_sb")
        nc.sync.dma_start(out=ones_sb[:], in_=ones_ext[:].bitcast(F32R))
        ident_sb = cpool.tile([128, 128], F32R, name="ident_sb", tag="ident_sb")
        nc.sync.dma_start(out=ident_sb[:], in_=ident_ext[:].bitcast(F32R))
        wo_t = []
        for h in range(HQ):
            wo_h = wpool.tile([128, HID], F32R, name=f"wo{h}", tag=f"wo{h}")
            nc.sync.dma_start(out=wo_h[:], in_=wo_ext[h * 128:(h + 1) * 128, :].bitcast(F32R))
            wo_t.append(wo_h)
        eps_sb = cpool.tile([1, 1], F32, name="eps_sb", tag="eps_sb")
        nc.vector.memset(eps_sb[:], EPS)

        kt_strips = []  # K-hat-T strips [128 d, STRIP tok], persistent
        v_tiles = []    # V tiles [128 tok, D], persistent
        pending_out = []

        def emit_out_proj(chunk=None):
            # chunk=None -> emit everything pending; chunk=tb -> emit one
            # tb-column of the oldest pending strip
            while pending_out:
                ps_, heads_ = pending_out[0]
                tbs = range(4) if chunk is None else [chunk]
                for tb in tbs:
                    tok0 = ps_ * STRIP + tb * 128
                    for hs in range(4):
                        op_ps = bigps.tile([128, STRIP], F32, name=f"op{ps_}_{tb}_{hs}", tag="bigps")
                        for h in range(HQ):
                            nc.tensor.matmul(
                                op_ps[:],
                                heads_[h][:, tb * 128:(tb + 1) * 128],
                                wo_t[h][:, hs * STRIP:(hs + 1) * STRIP],
                                start=(h == 0), stop=(h == HQ - 1),
                            )
                        ob = outp.tile([128, STRIP], F32, name=f"ob{ps_}_{tb}_{hs}", tag="ob")
                        nc.scalar.copy(ob[:], op_ps[:])
                        nc.sync.dma_start(
                            out=out_ext[tok0:tok0 + 128, hs * STRIP:(hs + 1) * STRIP],
                            in_=ob[:],
                        )
                if chunk is not None:
                    return
                pending_out.pop(0)

        def norm_row_scale(raw_ps, suffix):
            """[128, STRIP] psum -> [128, STRIP] bcast of 1/rms over partitions."""
            sq = scr.tile([128, STRIP], F32R, name=f"sq_{suffix}", tag="sq")
            nc.scalar.activation(sq[:], raw_ps[:], Square)
            ss_ps = rowps.tile([1, STRIP], F32, name=f"ss_{suffix}", tag="rowps")
            nc.tensor.matmul(ss_ps[:], ones_sb[:], sq[:], start=True, stop=True)
            rms = rowp.tile([1, STRIP], F32, name=f"rms_{suffix}", tag="rows")
            nc.scalar.activation(rms[:], ss_ps[:], Sqrt, bias=eps_sb[:], scale=1.0 / D)
            bcr = bcp.tile([128, STRIP], F32, name=f"bcr_{suffix}", tag="bc")
            nc.gpsimd.partition_broadcast(bcr[:], rms[:])
            bc = bcp.tile([128, STRIP], F32, name=f"bc_{suffix}", tag="bc")
            nc.vector.reciprocal_approx_fast(bc[:], bcr[:])
            return bc

        for s in range(NSTRIP):
            # ---- 1) load xT strip tiles [128 hid, STRIP tok] (x pre-transposed on host)
            tsl = slice(s * STRIP, (s + 1) * STRIP)
            if s == 0:
                xt = xt0
            else:
                xt = []
                for k in range(KT):
                    xt_k = xtp.tile([128, STRIP], F32R, name=f"xt{s}_{k}", tag="xt")
                    nc.sync.dma_start(out=xt_k[:], in_=xt_ext[k * 128:(k + 1) * 128, tsl].bitcast(F32R))
                    xt.append(xt_k)

            # ---- 2) projections: Q0, Q1, K, Q2, Q3, V. K's norm chain hides
            # under Q2/Q3 matmuls; V's matmuls + transposes cover Q3's chain.
            qt_h = [None] * HQ

            def project_q(h):
                qraw = bigps.tile([128, STRIP], F32, name=f"qraw{s}_{h}", tag="bigps")
                for k in range(KT):
                    nc.tensor.matmul(
                        qraw[:], wq_t[k][:, h * D:(h + 1) * D], xt[k][:],
                        start=(k == 0), stop=(k == KT - 1),
                    )
                bcq = norm_row_scale(qraw, f"q{s}_{h}")
                qn = qtp.tile([128, STRIP], F32R, name=f"qt{s}_{h}", tag="qt")
                nc.vector.scalar_tensor_tensor(qn[:], qraw[:], gq_sb[:], bcq[:], mult, mult)
                qt_h[h] = qn

            project_q(0)
            project_q(1)
            project_q(2)
            project_q(3)

            kraw = bigps.tile([128, STRIP], F32, name=f"kraw{s}", tag="bigps")
            for k in range(KT):
                nc.tensor.matmul(
                    kraw[:], wk_t[k][:], xt[k][:],
                    start=(k == 0), stop=(k == KT - 1),
                )
            bck = norm_row_scale(kraw, f"k{s}")
            kn = kvp.tile([128, STRIP], F32R, name=f"kt_strip{s}", tag="kt", bufs=NSTRIP)
            nc.vector.scalar_tensor_tensor(kn[:], kraw[:], gk_sb[:], bck[:], mult, mult)
            kt_strips.append(kn)

            vraw = bigps.tile([128, STRIP], F32, name=f"vraw{s}", tag="bigps")
            for k in range(KT):
                nc.tensor.matmul(
                    vraw[:], wv_t[k][:], xt[k][:],
                    start=(k == 0), stop=(k == KT - 1),
                )
            vt_sb = scr.tile([128, STRIP], F32R, name=f"vt_sb{s}", tag="sq")
            nc.vector.tensor_copy(vt_sb[:], vraw[:])
            for tb in range(4):
                tp = bigps.tile([128, 128], F32R, name=f"vtp{s}_{tb}", tag="bigps")
                nc.tensor.transpose(tp[:], vt_sb[:, tb * 128:(tb + 1) * 128], ident_sb[:])
                vt = kvp.tile([128, D], F32R, name=f"v{s}_{tb}", tag="v", bufs=TOKT)
                nc.vector.tensor_copy(vt[:], tp[:])
                v_tiles.append(vt)

            # ---- 3) attention for q-strip s, heads 0..3
            # (out-proj chunks of the previous strip are interleaved between
            # heads to fill PE bubbles in the softmax-bound phase)
            nkt = 4 * s + 4  # causal: k-tiles 0 .. 4s+3
            ot_heads = []
            for h in range(HQ):
                ot_ps = otps.tile([128, STRIP], F32, name=f"ot{s}_{h}", tag="otps")
                acc = accp.tile([128, STRIP], F32R, name=f"acc{s}_{h}", tag="acc")
                pts = [None] * nkt

                def issue_st(k, h=h, pts=pts):
                    st_ps = stps.tile([128, STRIP], F32, name=f"st{s}_{h}_{k}", tag="stps")
                    jj = k - 4 * s
                    # diagonal tiles: columns < 128*j fully masked — shrink the
                    # matmul to the live region (min N=256 keeps f32r full rate)
                    m0 = 0 if jj < 0 else min(128 * jj, STRIP - 256)
                    nc.tensor.matmul(
                        st_ps[:, m0:],
                        kt_strips[k // 4][:, (k % 4) * 128:(k % 4 + 1) * 128],
                        qt_h[h][:, m0:],
                        start=True, stop=True,
                    )
                    pt = ptp.tile([128, STRIP], F32R, name=f"pt{s}_{h}_{k}", tag="pt")
                    j = k - 4 * s
                    if j < 0:
                        nc.scalar.activation(pt[:], st_ps[:], Exp, scale=scale_qk)
                    else:
                        c0 = 128 * j
                        if c0 > 0:
                            nc.gpsimd.memset(pt[:, :c0].bitcast(F32), 0.0)
                        nc.scalar.activation(pt[:, c0:], st_ps[:, c0:], Exp, scale=scale_qk)
                        nc.vector.tensor_tensor(
                            pt[:, c0:c0 + 128], pt[:, c0:c0 + 128], tri_sb[:], mult
                        )
                    pts[k] = pt

                def issue_pv(k, ot_ps=ot_ps, acc=acc, pts=pts, nkt=nkt):
                    jj = k - 4 * s
                    m0 = 0 if (jj < 0 or k == 0) else min(128 * jj, STRIP - 256)
                    nc.tensor.matmul(
                        ot_ps[:, m0:], v_tiles[k][:], pts[k][:, m0:],
                        start=(k == 0), stop=(k == nkt - 1),
                    )
                    if k == 0:
                        nc.vector.tensor_copy(acc[:], pts[k][:])
                    else:
                        nc.vector.tensor_add(acc[:], acc[:], pts[k][:])

                # software-pipeline: ST(k+1) issued before PV(k)
                issue_st(0)
                for k in range(1, nkt):
                    issue_st(k)
                    issue_pv(k - 1)
                issue_pv(nkt - 1)

                # evict OT unnormalized immediately (frees the single otps bank),
                # normalize in place once the denominator row is ready
                ot_sb = otp.tile([128, STRIP], F32R, name=f"otsb{s}_{h}", tag="ot")
                nc.vector.tensor_copy(ot_sb[:], ot_ps[:])
                den_ps = rowps.tile([1, STRIP], F32, name=f"den{s}_{h}", tag="rowps")
                nc.tensor.matmul(den_ps[:], ones_sb[:], acc[:], start=True, stop=True)
                dstage = rowp.tile([1, STRIP], F32, name=f"dr{s}_{h}", tag="rows")
                nc.vector.tensor_copy(dstage[:], den_ps[:])
                bcd = bcp.tile([128, STRIP], F32, name=f"dbcr{s}_{h}", tag="bc")
                nc.gpsimd.partition_broadcast(bcd[:], dstage[:])
                bc = bcp.tile([128, STRIP], F32, name=f"dbc{s}_{h}", tag="bc")
                nc.vector.reciprocal_approx_fast(bc[:], bcd[:])
                nc.vector.tensor_tensor(ot_sb[:], ot_sb[:], bc[:], mult)
                ot_heads.append(ot_sb)
                if pending_out:
                    emit_out_proj(chunk=h)
                    if h == HQ - 1:
                        pending_out.pop(0)

            # ---- 4) output projection deferred into the next strip (fills the
            # softmax-tail PE bubble); emitted by emit_out_proj below.
            pending_out.append((s, ot_heads))

        emit_out_proj()

    nc.compile()
    return nc


_NC_CACHE = None
last_result = None


def _masks_np():
    # masks[0, :, :128] = lower-triangle 0/1 validity (kr <= qc)
    m = np.zeros((4, 128, STRIP), np.float32)
    kr = np.arange(128)[:, None]
    qc = np.arange(STRIP)[None, :]
    m[0] = np.where(kr <= qc, 1.0, 0.0)
    return m


def kernel(x, Wq, Wk, Wv, Wo, gq, gk):
    global _NC_CACHE, last_result
    x = np.asarray(x, np.float32)
    Wq = np.asarray(Wq, np.float32)
    Wk = np.asarray(Wk, np.float32)
    Wv = np.asarray(Wv, np.float32)
    Wo = np.asarray(Wo, np.float32)
    gq = np.asarray(gq, np.float32)
    gk = np.asarray(gk, np.float32)

    masks = _masks_np()
    ones = np.ones((128, 1), np.float32)
    ident = np.eye(128, dtype=np.float32)
    in_maps = []
    for core in range(NCORES):
        b, g = core // 4, core % 4
        in_maps.append({
            "xt": np.ascontiguousarray(x[b].T),
            "wq": np.ascontiguousarray(Wq[:, g * HQ * D:(g + 1) * HQ * D]),
            "wk": np.ascontiguousarray(Wk[:, g * D:(g + 1) * D]),
            "wv": np.ascontiguousarray(Wv[:, g * D:(g + 1) * D]),
            "wo": np.ascontiguousarray(Wo[g * HQ * D:(g + 1) * HQ * D, :]),
            "gq": np.ascontiguousarray(gq.reshape(D, 1)),
            "gk": np.ascontiguousarray(gk.reshape(D, 1)),
            "masks": masks,
            "ones": ones,
            "ident": ident,
        })

    if TRACE:
        _install_profile_shim()
    if _NC_CACHE is None:
        _NC_CACHE = build()
    last_result = run_bass_kernel_spmd(
        _NC_CACHE, in_maps, core_ids=list(range(NCORES)), trace=TRACE
    )
    out = np.zeros((2, S, HID), np.float32)
    for core in range(NCORES):
        out[core // 4] += last_result.results[core]["out"]
    return out
